# revision 1
# baseline (speedup 1.0000x reference)
"""MinGRU forward on 8 TRN2 NeuronCores.

Math (linear-space reformulation of the reference's log-space Heinsen scan):
    hg = x @ W_hg.T                       # [B,S,2D]
    hidden, gate = split(hg)
    z = sigmoid(gate)
    c = sigmoid(-gate)                    # = 1 - z = exp(-softplus(gate))
    g = max(hidden + 0.5, sigmoid(hidden))  # == where(h>=0, h+0.5, sigmoid(h)) exactly
    u = z * g
    h[t] = c[t] * h[t-1] + u[t]           # convex combination -> bounded, stable
    out = h

The recurrence maps directly onto the DVE `tensor_tensor_scan` instruction
(state = data0*state + data1 along the free dim, fp32 internal state).

Sharding: 8 cores = 4 batches x 2 feature-halves (512 features each).
No cross-core communication: the scan is per-feature independent.
Host pre-transposes x (-> xT [D,S]) and W (-> wT [D, 2*512]) so the kernel
needs no on-chip transposes; matmul uses fp32r (fp32 with 11-bit mantissa,
full-rate on the PE).  Inputs are pre-rounded to fp32r on the host (RNE).
"""

import numpy as np

B, S, D = 4, 4096, 1024
DH = D // 2          # features per core
N_CORES = 8
SC = 512             # tokens per seq chunk (PSUM bank = 512 fp32)
NSC = S // SC        # 8 seq chunks
KC = 128             # contraction chunk
NKC = D // KC        # 8 k chunks
FC = 128             # feature chunk (psum partitions)
NFC = DH // FC       # 4 feature chunks

_CACHE = {}

# build-time knobs (A/B tuning; defaults are the shipped config)
CONFIG = {
    "xbufs": 2,
    "psbufs": 4,
    "ebufs": 3,
    "xsplit": False,   # split x chunk DMAs per k-chunk
    "u_on_gpsimd": False,  # compute u = z*g on GpSimd instead of DVE
    "mm_interleave": False,  # alternate h/g matmuls per k-chunk
    "split_last_scan": True,  # last chunk: 2 chained half-scans so out-DMA overlaps
}


def _round_fp32r(a: np.ndarray) -> np.ndarray:
    """Round fp32 array to fp32r (11 explicit mantissa bits) with RNE."""
    u = np.ascontiguousarray(a, dtype=np.float32).view(np.uint32)
    r = (u + np.uint32(0x7FF) + ((u >> np.uint32(12)) & np.uint32(1))) & np.uint32(0xFFFFF000)
    return r.view(np.float32)


def _build():
    import concourse.bacc as bacc
    import concourse.tile as tile
    import concourse.mybir as mybir

    f32 = mybir.dt.float32
    f32r = mybir.dt.float32r
    AF = mybir.ActivationFunctionType
    OP = mybir.AluOpType

    nc = bacc.Bacc("TRN2")
    xT = nc.dram_tensor("xT", [D, S], f32r, kind="ExternalInput")
    # wT layout: [D, NFC, 2*FC] — per feature-chunk fc, 128 hidden cols then
    # 128 gate cols, contiguous, so each fc's weights are one 1 MiB DMA.
    wT = nc.dram_tensor("wT", [D, NFC, 2 * FC], f32r, kind="ExternalInput")
    outT = nc.dram_tensor("outT", [DH, S], f32, kind="ExternalOutput")

    with tile.TileContext(nc) as tc:
        with (
            tc.tile_pool(name="w", bufs=1) as wpool,
            tc.tile_pool(name="x", bufs=CONFIG["xbufs"]) as xpool,
            tc.tile_pool(name="ew", bufs=CONFIG["ebufs"]) as epool,
            tc.tile_pool(name="h", bufs=2) as hpool,
            tc.tile_pool(name="ps", bufs=CONFIG["psbufs"], space="PSUM") as pspool,
        ):
            # W loads on the ACT HWDGE ring (parallel with x on the SP ring).
            # Ring FIFO order matters: W fc0 first, then the second half of
            # x's first chunk (its first half rides the SP ring), then the
            # remaining W chunks — so the first psum's data all lands early.
            wts = []
            for fc in range(NFC):
                wtf = wpool.tile([KC, NKC, 2 * FC], f32r, tag=f"w{fc}")
                wts.append(wtf)

            def load_w(fc):
                nc.scalar.dma_start(
                    wts[fc][:], wT[:, fc, :].rearrange("(k p) e -> p k e", p=KC)
                )

            # Chunk widths: narrow at the start (softens the DMA ramp while W
            # streams in) and at the end (shortens the serial tail chain).
            widths = [512, 512, 512, 512, 512, 512, 512, 512]
            assert sum(widths) == S

            # Prefetch chunk 0 split across both rings, then chunk 1's second
            # half on the ACT ring between the remaining W loads.
            load_w(0)
            xt0 = xpool.tile([KC, NKC, widths[0]], f32r, tag="xt")
            xT_r0 = xT[:, 0:widths[0]].rearrange("(k p) s -> p k s", p=KC)
            # scalar ring: alternate x0 k-chunks with the remaining W chunks so
            # the first psum's x arrives early while W keeps streaming
            nc.scalar.dma_start(xt0[:, 4, :], xT_r0[:, 4, :])
            nc.scalar.dma_start(xt0[:, 5, :], xT_r0[:, 5, :])
            load_w(1)
            nc.scalar.dma_start(xt0[:, 6, :], xT_r0[:, 6, :])
            nc.scalar.dma_start(xt0[:, 7, :], xT_r0[:, 7, :])
            for k in range(NKC // 2):
                nc.sync.dma_start(xt0[:, k, :], xT_r0[:, k, :])
            load_w(2)
            load_w(3)

            hprev = [None] * NFC
            off = 0
            for sc, width in enumerate(widths):
                if sc == 0:
                    xt = xt0
                else:
                    xt = xpool.tile([KC, NKC, width], f32r, tag="xt")
                    xT_r = xT[:, off:off + width].rearrange("(k p) s -> p k s", p=KC)
                    if CONFIG["xsplit"]:
                        for k in range(NKC):
                            nc.sync.dma_start(xt[:, k, :], xT_r[:, k, :])
                    else:
                        nc.sync.dma_start(xt[:], xT_r)
                for fc in range(NFC):
                    ph = pspool.tile([FC, width], f32, tag="ph")
                    pg = pspool.tile([FC, width], f32, tag="pg")
                    if CONFIG["mm_interleave"]:
                        for k in range(NKC):
                            nc.tensor.matmul(
                                ph[:], wts[fc][:, k, 0:FC], xt[:, k, :],
                                start=(k == 0), stop=(k == NKC - 1),
                            )
                            nc.tensor.matmul(
                                pg[:], wts[fc][:, k, FC:2 * FC], xt[:, k, :],
                                start=(k == 0), stop=(k == NKC - 1),
                            )
                    else:
                        for k in range(NKC):
                            nc.tensor.matmul(
                                ph[:], wts[fc][:, k, 0:FC], xt[:, k, :],
                                start=(k == 0), stop=(k == NKC - 1),
                            )
                        for k in range(NKC):
                            nc.tensor.matmul(
                                pg[:], wts[fc][:, k, FC:2 * FC], xt[:, k, :],
                                start=(k == 0), stop=(k == NKC - 1),
                            )
                    zt = epool.tile([FC, width], f32, tag="z")
                    ct = epool.tile([FC, width], f32, tag="c")
                    st = epool.tile([FC, width], f32, tag="s")
                    gt = epool.tile([FC, width], f32, tag="g")
                    ut = epool.tile([FC, width], f32, tag="u")
                    # s first: it heads the DVE critical chain (s->g->u->scan)
                    nc.scalar.activation(st[:], ph[:], AF.Sigmoid)
                    nc.scalar.activation(zt[:], pg[:], AF.Sigmoid)
                    nc.scalar.activation(ct[:], pg[:], AF.Sigmoid, scale=-1.0)
                    # g = (hidden + 0.5) max sigmoid(hidden)
                    nc.vector.scalar_tensor_tensor(
                        gt[:], ph[:], 0.5, st[:], op0=OP.add, op1=OP.max
                    )
                    ueng = nc.gpsimd if CONFIG["u_on_gpsimd"] else nc.vector
                    ueng.tensor_mul(ut[:], zt[:], gt[:])
                    ht = hpool.tile([FC, width], f32, tag=f"h{fc}")
                    pw = widths[sc - 1]
                    init = 0.0 if sc == 0 else hprev[fc][:, pw - 1:pw]
                    if CONFIG["split_last_scan"] and sc == len(widths) - 1:
                        hw_ = width // 2
                        nc.vector.tensor_tensor_scan(
                            ht[:, 0:hw_], ct[:, 0:hw_], ut[:, 0:hw_], init,
                            op0=OP.mult, op1=OP.add,
                        )
                        nc.sync.dma_start(
                            outT[fc * FC:(fc + 1) * FC, off:off + hw_], ht[:, 0:hw_]
                        )
                        nc.vector.tensor_tensor_scan(
                            ht[:, hw_:width], ct[:, hw_:width], ut[:, hw_:width],
                            ht[:, hw_ - 1:hw_], op0=OP.mult, op1=OP.add,
                        )
                        nc.sync.dma_start(
                            outT[fc * FC:(fc + 1) * FC, off + hw_:off + width],
                            ht[:, hw_:width],
                        )
                        hprev[fc] = ht
                    else:
                        nc.vector.tensor_tensor_scan(
                            ht[:], ct[:], ut[:], init, op0=OP.mult, op1=OP.add
                        )
                        hprev[fc] = ht
                        nc.sync.dma_start(
                            outT[fc * FC:(fc + 1) * FC, off:off + width], ht[:]
                        )
                off += width

    nc.compile()
    return nc


def _prep_in_maps(x: np.ndarray, W_hg: np.ndarray):
    x = np.asarray(x, dtype=np.float32)
    W_hg = np.asarray(W_hg, dtype=np.float32)
    xTs = [_round_fp32r(np.ascontiguousarray(x[b].T)) for b in range(B)]
    wTs = []
    for c in range(2):
        # [D, NFC, 2*FC]: per fc, 128 hidden cols then 128 gate cols
        wt = np.empty((D, NFC, 2 * FC), dtype=np.float32)
        for fc in range(NFC):
            rows_h = W_hg[c * DH + fc * FC:c * DH + (fc + 1) * FC]      # [FC, D]
            rows_g = W_hg[D + c * DH + fc * FC:D + c * DH + (fc + 1) * FC]
            wt[:, fc, 0:FC] = rows_h.T
            wt[:, fc, FC:2 * FC] = rows_g.T
        wTs.append(_round_fp32r(wt))
    return [{"xT": xTs[core // 2], "wT": wTs[core % 2]} for core in range(N_CORES)]


def _get_runner():
    """Build the Bass module once and cache a compiled jax callable for it.

    Mirrors bass2jax.run_bass_via_pjrt's multi-core path, but keeps the
    jitted/sharded executable so repeat kernel() calls skip re-tracing.
    """
    if "runner" in _CACHE:
        return _CACHE["runner"]

    import jax
    from jax.experimental.shard_map import shard_map
    from jax.sharding import Mesh, PartitionSpec
    from concourse import bass2jax

    if "nc" not in _CACHE:
        _CACHE["nc"] = _build()
    nc = _CACHE["nc"]
    bass2jax.install_neuronx_cc_hook()

    in_names = ["xT", "wT"]
    out_name = "outT"
    out_shape, out_dtype = (DH, S), np.float32
    partition_name = nc.partition_id_tensor.name if nc.partition_id_tensor else None

    def _body(xT, wT, zout):
        operands = [xT, wT, zout]
        if partition_name is not None:
            operands.append(bass2jax.partition_id_tensor())
        outs = bass2jax._bass_exec_p.bind(
            *operands,
            out_avals=(jax.core.ShapedArray(out_shape, out_dtype),),
            in_names=tuple(in_names + [out_name] + ([partition_name] if partition_name else [])),
            out_names=(out_name,),
            lowering_input_output_aliases=(),
            sim_require_finite=True,
            sim_require_nnan=True,
            nc=nc,
        )
        return tuple(outs)

    devices = jax.devices()[:N_CORES]
    mesh = Mesh(np.asarray(devices), ("core",))
    sharded = jax.jit(
        shard_map(
            _body, mesh=mesh,
            in_specs=(PartitionSpec("core"),) * 3,
            out_specs=(PartitionSpec("core"),),
            check_rep=False,
        ),
        donate_argnums=(2,),
        keep_unused=True,
    )

    def run(in_maps):
        concat_x = np.concatenate([m["xT"] for m in in_maps], axis=0)
        concat_w = np.concatenate([m["wT"] for m in in_maps], axis=0)
        zeros = np.zeros((N_CORES * DH, S), np.float32)
        (out_arr,) = sharded(concat_x, concat_w, zeros)
        return np.asarray(out_arr).reshape(N_CORES, DH, S)

    _CACHE["runner"] = run
    return run


def kernel(x: np.ndarray, W_hg: np.ndarray) -> np.ndarray:
    run = _get_runner()
    in_maps = _prep_in_maps(x, W_hg)
    outs = run(in_maps)

    out = np.empty((B, S, D), dtype=np.float32)
    for core in range(N_CORES):
        b, c = core // 2, core % 2
        out[b, :, c * DH:(c + 1) * DH] = outs[core].T
    return out



# revision 2
# speedup vs baseline: 1.3182x; 1.3182x over previous
"""MinGRU forward on 8 TRN2 NeuronCores.

Math (linear-space reformulation of the reference's log-space Heinsen scan):
    hg = x @ W_hg.T                       # [B,S,2D]
    hidden, gate = split(hg)
    z = sigmoid(gate)
    c = sigmoid(-gate)                    # = 1 - z
    g = max(hidden + 0.5, sigmoid(hidden))  # == where(h>=0, h+0.5, sigmoid(h))
    u = z * g
    h[t] = c[t] * h[t-1] + u[t]           # convex combination -> bounded, stable
    out = h

Sharding: 8 cores = 4 batches x 2 feature-halves (512 features each).
No cross-core communication: the scan is per-feature independent.

All HBM I/O and matmul operands are fp16 (11 mantissa bits ~ fp32r's
host-rounded 11; verified max rel err ~2.4e-3 vs the 2e-2 gate) which halves
DMA traffic, SBUF footprint and LDWEIGHTS time vs fp32r at the same PE rate.
PSUM accumulation stays fp32; the scan's internal state is fp32.

Engine split per [128, width] tile:
  ACT:  st = sigmoid(ph), ct = sigmoid(-pg)            (2 ops, psum -> fp16)
  DVE:  gt = (ph + 0.5) max st ; zt = 1 - ct ; ut = zt*gt ; scan(ct, ut)
  PE:   16 fp16 matmuls (8 ph + 8 pg), psum fp32
  ACT ring: W loads (fc0 k-sliced for a fast first matmul)
  SP ring:  x in (chunk 0 k-sliced), h out (batched per chunk)
"""

import numpy as np

B, S, D = 4, 4096, 1024
DH = D // 2          # features per core
N_CORES = 8
KC = 128             # contraction chunk
NKC = D // KC        # 8 k chunks
FC = 128             # feature chunk (psum partitions)
NFC = DH // FC       # 4 feature chunks
WIDTHS = [512, 512, 512, 512, 512, 512, 512, 256, 256]
assert sum(WIDTHS) == S

_CACHE = {}


def _build():
    import concourse.bacc as bacc
    import concourse.tile as tile
    import concourse.mybir as mybir

    f32 = mybir.dt.float32
    f16 = mybir.dt.float16
    AF = mybir.ActivationFunctionType
    OP = mybir.AluOpType

    nc = bacc.Bacc("TRN2")
    xT = nc.dram_tensor("xT", [D, S], f16, kind="ExternalInput")
    # wT layout: [D, NFC, 2*FC] — per feature-chunk fc, 128 hidden cols then
    # 128 gate cols.
    wT = nc.dram_tensor("wT", [D, NFC, 2 * FC], f16, kind="ExternalInput")
    outT = nc.dram_tensor("outT", [DH, S], f16, kind="ExternalOutput")

    with tile.TileContext(nc) as tc:
        with (
            tc.tile_pool(name="w", bufs=1) as wpool,
            tc.tile_pool(name="x", bufs=3) as xpool,
            tc.tile_pool(name="ew", bufs=3) as epool,
            tc.tile_pool(name="h", bufs=2) as hpool,
            tc.tile_pool(name="ps", bufs=4, space="PSUM") as pspool,
        ):
            # All W rides the ACT ring; x and out ride the SP ring. fc0 is
            # k-sliced so the very first matmul only waits on 128KB, and the
            # fc-major consumption order matches W's arrival order.
            wts = []
            for fc in range(NFC):
                wtf = wpool.tile([KC, NKC, 2 * FC], f16, tag=f"w{fc}")
                wts.append(wtf)
            for k2 in range(NKC // 2):
                nc.scalar.dma_start(
                    wts[0][:, 2 * k2:2 * k2 + 2, :],
                    wT[k2 * 2 * KC:(k2 + 1) * 2 * KC, 0, :].rearrange(
                        "(k p) e -> p k e", p=KC
                    ),
                )
            for fc in range(1, NFC):
                nc.scalar.dma_start(
                    wts[fc][:], wT[:, fc, :].rearrange("(k p) e -> p k e", p=KC)
                )

            # x chunk 0 arrives in k-order to pace the first fc's matmuls.
            xt0 = xpool.tile([KC, NKC, WIDTHS[0]], f16, tag="xt")
            xT_r0 = xT[:, 0:WIDTHS[0]].rearrange("(k p) s -> p k s", p=KC)
            for k2 in range(NKC // 2):
                nc.sync.dma_start(
                    xt0[:, 2 * k2:2 * k2 + 2, :], xT_r0[:, 2 * k2:2 * k2 + 2, :]
                )

            hprev = None
            off = 0
            for sc, width in enumerate(WIDTHS):
                last = sc == len(WIDTHS) - 1
                if sc == 0:
                    xt = xt0
                else:
                    xt = xpool.tile([KC, NKC, width], f16, tag="xt")
                    xT_r = xT[:, off:off + width].rearrange("(k p) s -> p k s", p=KC)
                    nc.sync.dma_start(xt[:], xT_r)
                hall = hpool.tile([FC, NFC, width], f16, tag="hall")
                for fc in range(NFC):
                    ph = pspool.tile([FC, width], f32, tag="ph")
                    pg = pspool.tile([FC, width], f32, tag="pg")
                    for k in range(NKC):
                        nc.tensor.matmul(
                            ph[:], wts[fc][:, k, 0:FC], xt[:, k, :],
                            start=(k == 0), stop=(k == NKC - 1),
                        )
                    for k in range(NKC):
                        nc.tensor.matmul(
                            pg[:], wts[fc][:, k, FC:2 * FC], xt[:, k, :],
                            start=(k == 0), stop=(k == NKC - 1),
                        )
                    st = epool.tile([FC, width], f16, tag="s")
                    ct = epool.tile([FC, width], f16, tag="c")
                    zt = epool.tile([FC, width], f16, tag="z")
                    gt = epool.tile([FC, width], f16, tag="g")
                    ut = epool.tile([FC, width], f16, tag="u")
                    # st first: it heads the DVE critical chain (s->g->u->scan)
                    nc.scalar.activation(st[:], ph[:], AF.Sigmoid)
                    nc.scalar.activation(ct[:], pg[:], AF.Sigmoid, scale=-1.0)
                    # g = (hidden + 0.5) max sigmoid(hidden)
                    nc.vector.scalar_tensor_tensor(
                        gt[:], ph[:], 0.5, st[:], op0=OP.add, op1=OP.max
                    )
                    # z = 1 - c  (all-fp16 SBUF: fast DVE mode)
                    nc.vector.tensor_scalar(
                        zt[:], ct[:], -1.0, 1.0, op0=OP.mult, op1=OP.add
                    )
                    nc.vector.tensor_tensor(ut[:], zt[:], gt[:], op=OP.mult)
                    ho = hall[:, fc, :]
                    pw = WIDTHS[sc - 1]
                    init = 0.0 if sc == 0 else hprev[:, fc, pw - 1:pw]
                    if last:
                        # split the final scan so the out-DMA overlaps the rest
                        hw_ = width // 2
                        nc.vector.tensor_tensor_scan(
                            ho[:, 0:hw_], ct[:, 0:hw_], ut[:, 0:hw_], init,
                            op0=OP.mult, op1=OP.add,
                        )
                        nc.sync.dma_start(
                            outT[fc * FC:(fc + 1) * FC, off:off + hw_],
                            ho[:, 0:hw_],
                        )
                        nc.vector.tensor_tensor_scan(
                            ho[:, hw_:width], ct[:, hw_:width], ut[:, hw_:width],
                            hall[:, fc, hw_ - 1:hw_], op0=OP.mult, op1=OP.add,
                        )
                        nc.sync.dma_start(
                            outT[fc * FC:(fc + 1) * FC, off + hw_:off + width],
                            ho[:, hw_:width],
                        )
                    else:
                        nc.vector.tensor_tensor_scan(
                            ho[:], ct[:], ut[:], init, op0=OP.mult, op1=OP.add
                        )
                if not last:
                    nc.sync.dma_start(
                        outT[:, off:off + width].rearrange(
                            "(f p) s -> p f s", p=FC
                        ),
                        hall[:],
                    )
                hprev = hall
                off += width

    nc.compile()
    return nc


def _prep_in_maps(x: np.ndarray, W_hg: np.ndarray):
    x = np.asarray(x, dtype=np.float32)
    W_hg = np.asarray(W_hg, dtype=np.float32)
    xTs = [np.ascontiguousarray(x[b].T).astype(np.float16) for b in range(B)]
    wTs = []
    for c in range(2):
        # [D, NFC, 2*FC]: per fc, 128 hidden cols then 128 gate cols
        wt = np.empty((D, NFC, 2 * FC), dtype=np.float32)
        for fc in range(NFC):
            rows_h = W_hg[c * DH + fc * FC:c * DH + (fc + 1) * FC]      # [FC, D]
            rows_g = W_hg[D + c * DH + fc * FC:D + c * DH + (fc + 1) * FC]
            wt[:, fc, 0:FC] = rows_h.T
            wt[:, fc, FC:2 * FC] = rows_g.T
        wTs.append(wt.astype(np.float16))
    return [{"xT": xTs[core // 2], "wT": wTs[core % 2]} for core in range(N_CORES)]


def _get_runner():
    """Build the Bass module once and cache a compiled jax callable for it.

    Mirrors bass2jax.run_bass_via_pjrt's multi-core path, but keeps the
    jitted/sharded executable so repeat kernel() calls skip re-tracing.
    """
    if "runner" in _CACHE:
        return _CACHE["runner"]

    import jax
    from jax.experimental.shard_map import shard_map
    from jax.sharding import Mesh, PartitionSpec
    from concourse import bass2jax

    if "nc" not in _CACHE:
        _CACHE["nc"] = _build()
    nc = _CACHE["nc"]
    bass2jax.install_neuronx_cc_hook()

    in_names = ["xT", "wT"]
    out_name = "outT"
    out_shape, out_dtype = (DH, S), np.float16
    partition_name = nc.partition_id_tensor.name if nc.partition_id_tensor else None

    def _body(xT, wT, zout):
        operands = [xT, wT, zout]
        if partition_name is not None:
            operands.append(bass2jax.partition_id_tensor())
        outs = bass2jax._bass_exec_p.bind(
            *operands,
            out_avals=(jax.core.ShapedArray(out_shape, out_dtype),),
            in_names=tuple(in_names + [out_name] + ([partition_name] if partition_name else [])),
            out_names=(out_name,),
            lowering_input_output_aliases=(),
            sim_require_finite=True,
            sim_require_nnan=True,
            nc=nc,
        )
        return tuple(outs)

    devices = jax.devices()[:N_CORES]
    mesh = Mesh(np.asarray(devices), ("core",))
    sharded = jax.jit(
        shard_map(
            _body, mesh=mesh,
            in_specs=(PartitionSpec("core"),) * 3,
            out_specs=(PartitionSpec("core"),),
            check_rep=False,
        ),
        donate_argnums=(2,),
        keep_unused=True,
    )

    def run(in_maps):
        concat_x = np.concatenate([m["xT"] for m in in_maps], axis=0)
        concat_w = np.concatenate([m["wT"] for m in in_maps], axis=0)
        zeros = np.zeros((N_CORES * DH, S), np.float16)
        (out_arr,) = sharded(concat_x, concat_w, zeros)
        return np.asarray(out_arr).reshape(N_CORES, DH, S)

    _CACHE["runner"] = run
    return run


def kernel(x: np.ndarray, W_hg: np.ndarray) -> np.ndarray:
    run = _get_runner()
    in_maps = _prep_in_maps(x, W_hg)
    outs = run(in_maps)

    out = np.empty((B, S, D), dtype=np.float32)
    for core in range(N_CORES):
        b, c = core // 2, core % 2
        out[b, :, c * DH:(c + 1) * DH] = outs[core].T.astype(np.float32)
    return out


# revision 5
# speedup vs baseline: 1.3234x; 1.0040x over previous
"""MinGRU forward on 8 TRN2 NeuronCores.

Math (linear-space reformulation of the reference's log-space Heinsen scan):
    hg = x @ W_hg.T                       # [B,S,2D]
    hidden, gate = split(hg)
    z = sigmoid(gate)
    c = sigmoid(-gate)                    # = 1 - z
    g = max(hidden + 0.5, sigmoid(hidden))  # == where(h>=0, h+0.5, sigmoid(h))
    u = z * g
    h[t] = c[t] * h[t-1] + u[t]           # convex combination -> bounded, stable
    out = h

Sharding: 8 cores = 4 batches x 2 feature-halves (512 features each).
No cross-core communication: the scan is per-feature independent.

All HBM I/O and matmul operands are fp16 (11 mantissa bits ~ fp32r's
host-rounded 11; verified max rel err ~2.4e-3 vs the 2e-2 gate) which halves
DMA traffic, SBUF footprint and LDWEIGHTS time vs fp32r at the same PE rate.
PSUM accumulation stays fp32; the scan's internal state is fp32.

Engine split per [128, width] tile:
  ACT:  st = sigmoid(ph), ct = sigmoid(-pg)            (2 ops, psum -> fp16)
  DVE:  gt = (ph + 0.5) max st ; zt = 1 - ct ; ut = zt*gt ; scan(ct, ut)
  PE:   16 fp16 matmuls (8 ph + 8 pg), psum fp32
  ACT ring: W loads (fc0 k-sliced for a fast first matmul)
  SP ring:  x in (chunk 0 k-sliced), h out (batched per chunk)
"""

import numpy as np

B, S, D = 4, 4096, 1024
DH = D // 2          # features per core
N_CORES = 8
KC = 128             # contraction chunk
NKC = D // KC        # 8 k chunks
FC = 128             # feature chunk (psum partitions)
NFC = DH // FC       # 4 feature chunks
WIDTHS = [512, 512, 512, 512, 512, 512, 512, 256, 256]
assert sum(WIDTHS) == S

_CACHE = {}


def _build():
    import concourse.bacc as bacc
    import concourse.tile as tile
    import concourse.mybir as mybir

    f32 = mybir.dt.float32
    f16 = mybir.dt.float16
    AF = mybir.ActivationFunctionType
    OP = mybir.AluOpType

    nc = bacc.Bacc("TRN2")
    xT = nc.dram_tensor("xT", [D, S], f16, kind="ExternalInput")
    # wT layout: [D, NFC, 2*FC] — per feature-chunk fc, 128 hidden cols then
    # 128 gate cols.
    wT = nc.dram_tensor("wT", [D, NFC, 2 * FC], f16, kind="ExternalInput")
    outT = nc.dram_tensor("outT", [DH, S], f16, kind="ExternalOutput")

    with tile.TileContext(nc) as tc:
        with (
            tc.tile_pool(name="w", bufs=1) as wpool,
            tc.tile_pool(name="x", bufs=3) as xpool,
            tc.tile_pool(name="ew", bufs=3) as epool,
            tc.tile_pool(name="h", bufs=2) as hpool,
            tc.tile_pool(name="ps", bufs=4, space="PSUM") as pspool,
        ):
            # All W rides the ACT ring; x and out ride the SP ring. fc0 is
            # k-sliced (finest first) so the very first matmul only waits on
            # 64KB, and the fc-major consumption order matches W's arrival
            # order.
            wts = []
            for fc in range(NFC):
                wtf = wpool.tile([KC, NKC, 2 * FC], f16, tag=f"w{fc}")
                wts.append(wtf)

            def load_w(fc, k0, k1):
                nc.scalar.dma_start(
                    wts[fc][:, k0:k1, :],
                    wT[k0 * KC:k1 * KC, fc, :].rearrange("(k p) e -> p k e", p=KC),
                )

            for k0, k1 in [(0, 1), (1, 2), (2, 4), (4, 6), (6, 8)]:
                load_w(0, k0, k1)
            load_w(1, 0, 4)
            load_w(1, 4, 8)
            load_w(2, 0, 8)
            load_w(3, 0, 8)

            # x chunk 0 arrives in k-order to pace the first fc's matmuls.
            xt0 = xpool.tile([KC, NKC, WIDTHS[0]], f16, tag="xt")
            xT_r0 = xT[:, 0:WIDTHS[0]].rearrange("(k p) s -> p k s", p=KC)
            for k0, k1 in [(0, 1), (1, 2), (2, 4), (4, 6), (6, 8)]:
                nc.sync.dma_start(xt0[:, k0:k1, :], xT_r0[:, k0:k1, :])

            # Warm-up matmuls on a zeroed scratch tile: the PE p-state ramps
            # to full clock only after ~3us of continuous execution, and the
            # PE would otherwise sit idle until the first W slice lands
            # (~4.5us of DMA-queue spin-up).  These overlap that dead time
            # so the real matmul stream starts at full speed.
            wwarm = wpool.tile([KC, 512], f16, tag="warm")
            nc.gpsimd.memset(wwarm[:], 0.0)
            for _ in range(8):
                pwarm = pspool.tile([FC, 512], f32, tag="ph")
                nc.tensor.matmul(
                    pwarm[:], wwarm[:, 0:FC], wwarm[:], start=True, stop=True
                )

            hprev = None
            off = 0
            for sc, width in enumerate(WIDTHS):
                last = sc == len(WIDTHS) - 1
                if sc == 0:
                    xt = xt0
                else:
                    xt = xpool.tile([KC, NKC, width], f16, tag="xt")
                    xT_r = xT[:, off:off + width].rearrange("(k p) s -> p k s", p=KC)
                    nc.sync.dma_start(xt[:], xT_r)
                hall = hpool.tile([FC, NFC, width], f16, tag="hall")
                for fc in range(NFC):
                    ph = pspool.tile([FC, width], f32, tag="ph")
                    pg = pspool.tile([FC, width], f32, tag="pg")
                    if sc == 0:
                        # Interleave ph/pg per k-pair so each arriving W/x
                        # slice feeds 4 matmuls before the next is needed —
                        # keeps the PE fed while W streams in.
                        for k2 in range(NKC // 2):
                            for k in (2 * k2, 2 * k2 + 1):
                                nc.tensor.matmul(
                                    ph[:], wts[fc][:, k, 0:FC], xt[:, k, :],
                                    start=(k == 0), stop=(k == NKC - 1),
                                )
                            for k in (2 * k2, 2 * k2 + 1):
                                nc.tensor.matmul(
                                    pg[:], wts[fc][:, k, FC:2 * FC], xt[:, k, :],
                                    start=(k == 0), stop=(k == NKC - 1),
                                )
                    else:
                        for k in range(NKC):
                            nc.tensor.matmul(
                                ph[:], wts[fc][:, k, 0:FC], xt[:, k, :],
                                start=(k == 0), stop=(k == NKC - 1),
                            )
                        for k in range(NKC):
                            nc.tensor.matmul(
                                pg[:], wts[fc][:, k, FC:2 * FC], xt[:, k, :],
                                start=(k == 0), stop=(k == NKC - 1),
                            )
                    st = epool.tile([FC, width], f16, tag="s")
                    ct = epool.tile([FC, width], f16, tag="c")
                    zt = epool.tile([FC, width], f16, tag="z")
                    gt = epool.tile([FC, width], f16, tag="g")
                    ut = epool.tile([FC, width], f16, tag="u")
                    # st first: it heads the DVE critical chain (s->g->u->scan)
                    nc.scalar.activation(st[:], ph[:], AF.Sigmoid)
                    nc.scalar.activation(ct[:], pg[:], AF.Sigmoid, scale=-1.0)
                    # g = (hidden + 0.5) max sigmoid(hidden)
                    nc.vector.scalar_tensor_tensor(
                        gt[:], ph[:], 0.5, st[:], op0=OP.add, op1=OP.max
                    )
                    # z = 1 - c  (all-fp16 SBUF: fast DVE mode)
                    nc.vector.tensor_scalar(
                        zt[:], ct[:], -1.0, 1.0, op0=OP.mult, op1=OP.add
                    )
                    nc.vector.tensor_tensor(ut[:], zt[:], gt[:], op=OP.mult)
                    ho = hall[:, fc, :]
                    pw = WIDTHS[sc - 1]
                    init = 0.0 if sc == 0 else hprev[:, fc, pw - 1:pw]
                    if last and fc == NFC - 1:
                        # The very last feature chunk is the serial tail:
                        # split its scan so the first half's out-DMA overlaps
                        # the second half.
                        hw_ = width // 2
                        nc.vector.tensor_tensor_scan(
                            ho[:, 0:hw_], ct[:, 0:hw_], ut[:, 0:hw_], init,
                            op0=OP.mult, op1=OP.add,
                        )
                        nc.sync.dma_start(
                            outT[fc * FC:(fc + 1) * FC, off:off + hw_],
                            ho[:, 0:hw_],
                        )
                        nc.vector.tensor_tensor_scan(
                            ho[:, hw_:width], ct[:, hw_:width], ut[:, hw_:width],
                            hall[:, fc, hw_ - 1:hw_], op0=OP.mult, op1=OP.add,
                        )
                        nc.sync.dma_start(
                            outT[fc * FC:(fc + 1) * FC, off + hw_:off + width],
                            ho[:, hw_:width],
                        )
                    elif last:
                        nc.vector.tensor_tensor_scan(
                            ho[:], ct[:], ut[:], init, op0=OP.mult, op1=OP.add
                        )
                        nc.sync.dma_start(
                            outT[fc * FC:(fc + 1) * FC, off:off + width], ho[:]
                        )
                    else:
                        nc.vector.tensor_tensor_scan(
                            ho[:], ct[:], ut[:], init, op0=OP.mult, op1=OP.add
                        )
                if not last:
                    nc.sync.dma_start(
                        outT[:, off:off + width].rearrange(
                            "(f p) s -> p f s", p=FC
                        ),
                        hall[:],
                    )
                hprev = hall
                off += width

    nc.compile()
    return nc


def _prep_in_maps(x: np.ndarray, W_hg: np.ndarray):
    x = np.asarray(x, dtype=np.float32)
    W_hg = np.asarray(W_hg, dtype=np.float32)
    xTs = [np.ascontiguousarray(x[b].T).astype(np.float16) for b in range(B)]
    wTs = []
    for c in range(2):
        # [D, NFC, 2*FC]: per fc, 128 hidden cols then 128 gate cols
        wt = np.empty((D, NFC, 2 * FC), dtype=np.float32)
        for fc in range(NFC):
            rows_h = W_hg[c * DH + fc * FC:c * DH + (fc + 1) * FC]      # [FC, D]
            rows_g = W_hg[D + c * DH + fc * FC:D + c * DH + (fc + 1) * FC]
            wt[:, fc, 0:FC] = rows_h.T
            wt[:, fc, FC:2 * FC] = rows_g.T
        wTs.append(wt.astype(np.float16))
    return [{"xT": xTs[core // 2], "wT": wTs[core % 2]} for core in range(N_CORES)]


def _get_runner():
    """Build the Bass module once and cache a compiled jax callable for it.

    Mirrors bass2jax.run_bass_via_pjrt's multi-core path, but keeps the
    jitted/sharded executable so repeat kernel() calls skip re-tracing.
    """
    if "runner" in _CACHE:
        return _CACHE["runner"]

    import jax
    from jax.experimental.shard_map import shard_map
    from jax.sharding import Mesh, PartitionSpec
    from concourse import bass2jax

    if "nc" not in _CACHE:
        _CACHE["nc"] = _build()
    nc = _CACHE["nc"]
    bass2jax.install_neuronx_cc_hook()

    in_names = ["xT", "wT"]
    out_name = "outT"
    out_shape, out_dtype = (DH, S), np.float16
    partition_name = nc.partition_id_tensor.name if nc.partition_id_tensor else None

    def _body(xT, wT, zout):
        operands = [xT, wT, zout]
        if partition_name is not None:
            operands.append(bass2jax.partition_id_tensor())
        outs = bass2jax._bass_exec_p.bind(
            *operands,
            out_avals=(jax.core.ShapedArray(out_shape, out_dtype),),
            in_names=tuple(in_names + [out_name] + ([partition_name] if partition_name else [])),
            out_names=(out_name,),
            lowering_input_output_aliases=(),
            sim_require_finite=True,
            sim_require_nnan=True,
            nc=nc,
        )
        return tuple(outs)

    devices = jax.devices()[:N_CORES]
    mesh = Mesh(np.asarray(devices), ("core",))
    sharded = jax.jit(
        shard_map(
            _body, mesh=mesh,
            in_specs=(PartitionSpec("core"),) * 3,
            out_specs=(PartitionSpec("core"),),
            check_rep=False,
        ),
        donate_argnums=(2,),
        keep_unused=True,
    )

    def run(in_maps):
        concat_x = np.concatenate([m["xT"] for m in in_maps], axis=0)
        concat_w = np.concatenate([m["wT"] for m in in_maps], axis=0)
        zeros = np.zeros((N_CORES * DH, S), np.float16)
        (out_arr,) = sharded(concat_x, concat_w, zeros)
        return np.asarray(out_arr).reshape(N_CORES, DH, S)

    _CACHE["runner"] = run
    return run


def kernel(x: np.ndarray, W_hg: np.ndarray) -> np.ndarray:
    run = _get_runner()
    in_maps = _prep_in_maps(x, W_hg)
    outs = run(in_maps)

    out = np.empty((B, S, D), dtype=np.float32)
    for core in range(N_CORES):
        b, c = core // 2, core % 2
        out[b, :, c * DH:(c + 1) * DH] = outs[core].T.astype(np.float32)
    return out
